# revision 27
# baseline (speedup 1.0000x reference)
"""Distributed Trainium2 kernel for the dense-graph GNN layer.

Math: with xn = x/||x|| (rows), G = xn@xn.T, d = rsqrt(G@1),
out = (diag(d) G diag(d) x) W.  The N x N Gram matrix is never needed:
  t = colsum(xn) = X^T invn                                   [D]
  r_i = x_i . t ;  f_i = rsqrt(||x_i|| * r_i)
  z = X^T diag(f) X                                           [D, D]
  out = f_loc * (X_loc @ (z @ W))

Distribution: collectives in this environment cost ~40us+ each (measured:
both AllGather and AllReduce stall every core for tens of us in the
axon/fake_nrt runtime), so this kernel uses ZERO collectives.  Every core
receives the FULL x (rolled so its own 1024 rows come first - the global
reductions t and z are permutation-invariant) and redundantly computes the
global reductions, then produces only its local 1024-row output slice.
Per-core cost is dominated by streaming the 8MB x from HBM once.

Row layout: within each 1024-row chunk, partition p holds rows 8p..8p+7
(8KB contiguous per partition per chunk -> efficient DMA descriptors).
All global reductions are row-permutation-invariant; the local output
store inverts the same mapping.
"""

import os
import sys

import numpy as np

for _p in ("/opt/trn_rl_repo", "/root/.axon_site/_ro/trn_rl_repo"):
    if os.path.isdir(_p) and _p not in sys.path:
        sys.path.insert(0, _p)

import concourse.bacc as bacc
import concourse.mybir as mybir
import concourse.tile as tile
import concourse.masks as masks
from concourse import bass_utils

R = 8                  # cores
N, D = 8192, 256
NL = N // R            # 1024 rows per core (local shard)
P = 128
T = N // P             # 64 row tiles per core (full x)
TL = NL // P           # 8 local row tiles
CH = 8                 # tiles per DMA chunk
NCH = T // CH          # 8 chunks
CW = CH * D            # chunk width in columns (2048)
F32 = mybir.dt.float32
BF16 = mybir.dt.bfloat16
AF = mybir.ActivationFunctionType
ALU = mybir.AluOpType
AX = mybir.AxisListType

_cache = {}


def _rowsum(nc, out_col, ch3, scratch3):
    """Row-sum of ch3 [128, CH, 256] -> out_col [128, CH].
    Three 2x halving adds into scratch, then a 1x reduce of [128, CH, 32]."""
    w = 128
    nc.vector.tensor_add(scratch3[:, :, 0:w], ch3[:, :, 0:w], ch3[:, :, w:2 * w])
    while w > 32:
        h = w // 2
        nc.vector.tensor_add(scratch3[:, :, 0:h], scratch3[:, :, 0:h],
                             scratch3[:, :, h:2 * h])
        w = h
    nc.vector.tensor_reduce(out_col, scratch3[:, :, 0:w], axis=AX.X, op=ALU.add)


def _program(tc, x, W, out):
    nc = tc.nc
    with (
        tc.tile_pool(name="persist", bufs=1) as pp,
        tc.tile_pool(name="xin", bufs=2) as xp,
        tc.tile_pool(name="work", bufs=3) as wp,
        tc.tile_pool(name="psA", bufs=1, space="PSUM") as psA,
        tc.tile_pool(name="psW", bufs=4, space="PSUM") as psW,
    ):
        xb_all = pp.tile([P, T * D], BF16)       # bf16 x, resident (4MB)
        nsq = pp.tile([P, T], BF16)              # row sum-of-squares
        nrm = pp.tile([P, T], F32)               # ||x_i||
        invn = pp.tile([P, T], F32)
        invn_bf = pp.tile([P, T], BF16)
        r_f = pp.tile([P, T], F32)               # x_i . t
        p_t = pp.tile([P, T], F32)
        sp_t = pp.tile([P, T], F32)
        f_t = pp.tile([P, T], F32)               # f = rsqrt(nrm * r)

        W_sb = pp.tile([P, 2 * D], F32)          # W k-chunk kc at [:, kc*D]
        Wb_sb = pp.tile([P, 2 * D], BF16)
        t_sb = pp.tile([1, D], BF16)
        ones_bf = pp.tile([1, P], BF16)
        tb_sb = pp.tile([P, D], BF16)            # t broadcast to 128 partitions
        ident_f = pp.tile([P, P], F32)
        ident_bf = pp.tile([P, P], BF16)
        xbT = pp.tile([P, 2 * NL], BF16)         # local x^T: block (i,c) at (2i+c)*P
        z_top_sb = pp.tile([P, D], BF16)         # [z11 | z12]
        z22_sb = pp.tile([P, P], BF16)
        z21_sb = pp.tile([P, P], BF16)
        zw_sb = pp.tile([P, 2 * D], BF16)        # zw rows 0:128 at [:,0:D], 128:256 at [:,D:2D]

        t_ps = psA.tile([1, D], F32, name="t_ps")
        tb_ps = psA.tile([P, D], F32, name="tb_ps")
        z_top_ps = psA.tile([P, D], F32, name="z_top_ps")
        z22_ps = psA.tile([P, P], F32, name="z22_ps")

        # x chunk DMAs first - everything else hides under them
        x_chs = []
        for c in range(NCH):
            x_ch = xp.tile([P, CW], F32, tag="xch", name=f"xch{c}")
            src = x[c * CH * P:(c + 1) * CH * P, :].rearrange(
                "(p j) d -> p j d", p=P
            )
            nc.sync.dma_start(x_ch[:].rearrange("p (j d) -> p j d", j=CH), src)
            x_chs.append(x_ch)

        for kc in range(2):
            nc.sync.dma_start(W_sb[:, kc * D:(kc + 1) * D], W[kc * P:(kc + 1) * P, :])

        masks.make_identity(nc, ident_f[:])
        nc.vector.tensor_copy(ident_bf[:], ident_f[:])
        nc.gpsimd.memset(ones_bf[:], 1.0)
        nc.scalar.copy(Wb_sb[:], W_sb[:])
        # preload both activation table sets during the DMA lead-in so the
        # ~1.3us ACT_TABLE_LOADs stay off the pass-1 critical chain
        warm = pp.tile([1, 8], F32)
        nc.scalar.activation(warm[:], ident_f[0:1, 0:8], AF.Square)
        nc.scalar.activation(warm[:], ident_f[0:1, 0:8], AF.Sqrt)

        # ---- pass 1 (overlapped with DMA): cast, row sumsq, t accumulation ----
        for c in range(NCH):
            x_ch = x_chs[c]
            cs = slice(c * CH, (c + 1) * CH)
            xb_ch = xb_all[:, c * CW:(c + 1) * CW]
            nc.vector.tensor_copy(xb_ch, x_ch[:])
            sq_ch = wp.tile([P, CW], BF16, tag="sq", name=f"sq{c}")
            nc.scalar.activation(sq_ch[:], x_ch[:], AF.Square)
            _rowsum(nc, nsq[:, cs], sq_ch[:].rearrange("p (t d) -> p t d", t=CH),
                    sq_ch[:].rearrange("p (t d) -> p t d", t=CH))
            nc.scalar.activation(nrm[:, cs], nsq[:, cs], AF.Sqrt)
            nc.vector.reciprocal(invn[:, cs], nrm[:, cs])
            nc.vector.tensor_copy(invn_bf[:, cs], invn[:, cs])

            if c == 0:
                # local x^T for the final GEMM - PE is idle during the load
                for i in range(TL):
                    for h in range(2):
                        pt = psW.tile([P, P], BF16, tag="pw", name=f"pt{i}_{h}")
                        nc.tensor.transpose(
                            pt[:], xb_all[:, i * D + h * P:i * D + (h + 1) * P],
                            ident_bf[:],
                        )
                        nc.scalar.copy(
                            xbT[:, (2 * i + h) * P:(2 * i + h + 1) * P], pt[:]
                        )

            for i in range(c * CH, (c + 1) * CH):
                nc.tensor.matmul(
                    t_ps[:], lhsT=invn_bf[:, i:i + 1],
                    rhs=xb_all[:, i * D:(i + 1) * D],
                    start=(i == 0), stop=(i == T - 1),
                )

        # ---- barrier: t ready; broadcast to 128 partitions ----
        nc.scalar.copy(t_sb[:], t_ps[:])
        nc.tensor.matmul(tb_ps[:], lhsT=ones_bf[:], rhs=t_sb[:], start=True, stop=True)
        nc.vector.tensor_copy(tb_sb[:], tb_ps[:])

        # ---- pass 2: r = x.t, f, g = f*x, z accumulation (PE) ----
        for c in range(NCH):
            cs = slice(c * CH, (c + 1) * CH)
            xb3 = xb_all[:, c * CW:(c + 1) * CW].rearrange("p (t d) -> p t d", t=CH)
            u_ch = wp.tile([P, D], BF16, tag="u", name=f"u{c}")
            # fused (xb * tb) + rowsum in one DVE op per tile; u_ch is a
            # rotating junk buffer for the mandatory full-tensor output
            for j in range(CH):
                i = c * CH + j
                nc.vector.scalar_tensor_tensor(
                    u_ch[:], xb_all[:, i * D:(i + 1) * D], 0.0, tb_sb[:],
                    op0=ALU.bypass, op1=ALU.mult,
                    accum_out=r_f[:, i:i + 1],
                )
            nc.vector.tensor_mul(p_t[:, cs], nrm[:, cs], r_f[:, cs])
            nc.scalar.activation(sp_t[:, cs], p_t[:, cs], AF.Sqrt)
            nc.vector.reciprocal(f_t[:, cs], sp_t[:, cs])

            g_ch = wp.tile([P, CW], BF16, tag="g", name=f"g{c}")
            for j in range(CH):
                i = c * CH + j
                g_i = g_ch[:, j * D:(j + 1) * D]
                if j % 2 == 0:
                    nc.scalar.mul(g_i, xb_all[:, i * D:(i + 1) * D],
                                  f_t[:, i:i + 1])
                else:
                    nc.vector.tensor_scalar_mul(g_i, xb_all[:, i * D:(i + 1) * D],
                                                f_t[:, i:i + 1])
                nc.tensor.matmul(
                    z_top_ps[:], lhsT=xb_all[:, i * D:i * D + P], rhs=g_i,
                    start=(i == 0), stop=(i == T - 1),
                )
                nc.tensor.matmul(
                    z22_ps[:], lhsT=xb_all[:, i * D + P:(i + 1) * D],
                    rhs=g_ch[:, j * D + P:(j + 1) * D],
                    start=(i == 0), stop=(i == T - 1),
                )

        # ---- zw = z @ W using symmetry (z21 = z12^T) ----
        nc.scalar.copy(z_top_sb[:], z_top_ps[:])
        nc.vector.tensor_copy(z22_sb[:], z22_ps[:])
        zT_ps = psW.tile([P, P], BF16, tag="pw", name="zT")
        nc.tensor.transpose(zT_ps[:], z_top_sb[:, P:D], ident_bf[:])
        nc.vector.tensor_copy(z21_sb[:], zT_ps[:])

        for half, (lhs1, lhs2) in enumerate(
            ((z_top_sb[:, 0:P], z21_sb[:]),        # zw_top = z11 W1 + (z12^T)^T W2
             (z_top_sb[:, P:D], z22_sb[:]))        # zw_bot = z12^T W1 + z22 W2
        ):
            zw_ps = psW.tile([P, D], F32, tag="pw", name=f"zw{half}")
            nc.tensor.matmul(zw_ps[:], lhsT=lhs1, rhs=Wb_sb[:, 0:D],
                             start=True, stop=False)
            nc.tensor.matmul(zw_ps[:], lhsT=lhs2, rhs=Wb_sb[:, D:2 * D],
                             start=False, stop=True)
            nc.vector.tensor_copy(zw_sb[:, half * D:(half + 1) * D], zw_ps[:])

        # ---- final: out_j = f_j * (x_j @ zw) for the 8 local tiles ----
        out3 = out.rearrange("(p j) d -> p j d", p=P)
        for i in range(TL):
            o_ps = psW.tile([P, D], F32, tag="pw", name=f"o{i}")
            for h in range(2):
                nc.tensor.matmul(
                    o_ps[:], lhsT=xbT[:, (2 * i + h) * P:(2 * i + h + 1) * P],
                    rhs=zw_sb[:, h * D:(h + 1) * D],
                    start=(h == 0), stop=(h == 1),
                )
            o_sb = wp.tile([P, D], F32, tag="osb", name=f"osb{i}")
            if i % 2 == 0:
                nc.scalar.mul(o_sb[:], o_ps[:], f_t[:, i:i + 1])
            else:
                nc.vector.tensor_scalar_mul(o_sb[:], o_ps[:], f_t[:, i:i + 1])
            nc.sync.dma_start(out3[:, i, :], o_sb[:])


def _build():
    nc = bacc.Bacc("TRN2", target_bir_lowering=False, debug=False, num_devices=R)
    x = nc.dram_tensor("x", [N, D], F32, kind="ExternalInput")
    W = nc.dram_tensor("W", [D, D], F32, kind="ExternalInput")
    out = nc.dram_tensor("out", [NL, D], F32, kind="ExternalOutput")
    with nc.allow_low_precision("bf16 row reductions; validated ~1.6e-3 rel err"):
        with tile.TileContext(nc) as tc:
            _program(
                tc,
                x.ap() if hasattr(x, "ap") else x,
                W.ap() if hasattr(W, "ap") else W,
                out.ap() if hasattr(out, "ap") else out,
            )
    nc.finalize()
    return nc


def _run(inputs, trace=False):
    if "nc" not in _cache:
        _cache["nc"] = _build()
    nc = _cache["nc"]
    x = np.ascontiguousarray(inputs["x"], dtype=np.float32)
    W = np.ascontiguousarray(inputs["W"], dtype=np.float32)
    in_maps = [
        {"x": np.roll(x, -r * NL, axis=0), "W": W} for r in range(R)
    ]
    res = bass_utils.run_bass_kernel_spmd(
        nc, in_maps, core_ids=list(range(R)), trace=trace,
    )
    out = np.concatenate([res.results[r]["out"] for r in range(R)], axis=0)
    return out, res


def kernel(**inputs) -> np.ndarray:
    out, _ = _run(inputs, trace=False)
    return out


# revision 29
# speedup vs baseline: 1.0665x; 1.0665x over previous
"""Distributed Trainium2 kernel for the dense-graph GNN layer.

Math: with xn = x/||x|| (rows), G = xn@xn.T, d = rsqrt(G@1),
out = (diag(d) G diag(d) x) W.  The N x N Gram matrix is never needed:
  t = colsum(xn) = X^T invn                                   [D]
  r_i = x_i . t ;  f_i = rsqrt(||x_i|| * r_i)
  z = X^T diag(f) X                                           [D, D]
  out = f_loc * (X_loc @ (z @ W))

Distribution: collectives in this environment cost ~40us+ each (measured:
both AllGather and AllReduce stall every core for tens of us in the
axon/fake_nrt runtime), so this kernel uses ZERO collectives.  Every core
receives the FULL x (rolled so its own 1024 rows come first - the global
reductions t and z are permutation-invariant) and redundantly computes the
global reductions, then produces only its local 1024-row output slice.
Per-core cost is dominated by streaming the 8MB x from HBM once.

Row layout: within each 1024-row chunk, partition p holds rows 8p..8p+7
(8KB contiguous per partition per chunk -> efficient DMA descriptors).
All global reductions are row-permutation-invariant; the local output
store inverts the same mapping.
"""

import os
import sys

import numpy as np

for _p in ("/opt/trn_rl_repo", "/root/.axon_site/_ro/trn_rl_repo"):
    if os.path.isdir(_p) and _p not in sys.path:
        sys.path.insert(0, _p)

import concourse.bacc as bacc
import concourse.mybir as mybir
import concourse.tile as tile
import concourse.masks as masks
from concourse import bass_utils

R = 8                  # cores
N, D = 8192, 256
NL = N // R            # 1024 rows per core (local shard)
P = 128
T = N // P             # 64 row tiles per core (full x)
TL = NL // P           # 8 local row tiles
CH = 8                 # tiles per DMA chunk
NCH = T // CH          # 8 chunks
CW = CH * D            # chunk width in columns (2048)
F32 = mybir.dt.float32
BF16 = mybir.dt.bfloat16
AF = mybir.ActivationFunctionType
ALU = mybir.AluOpType
AX = mybir.AxisListType

_cache = {}


def _rowsum(nc, out_col, ch3, scratch3):
    """Row-sum of ch3 [128, CH, 256] -> out_col [128, CH].
    Three 2x halving adds into scratch, then a 1x reduce of [128, CH, 32]."""
    w = 128
    nc.vector.tensor_add(scratch3[:, :, 0:w], ch3[:, :, 0:w], ch3[:, :, w:2 * w])
    while w > 32:
        h = w // 2
        nc.vector.tensor_add(scratch3[:, :, 0:h], scratch3[:, :, 0:h],
                             scratch3[:, :, h:2 * h])
        w = h
    nc.vector.tensor_reduce(out_col, scratch3[:, :, 0:w], axis=AX.X, op=ALU.add)


def _program(tc, x, W, out):
    nc = tc.nc
    with (
        tc.tile_pool(name="persist", bufs=1) as pp,
        tc.tile_pool(name="xin", bufs=3) as xp,
        tc.tile_pool(name="work", bufs=3) as wp,
        tc.tile_pool(name="psA", bufs=1, space="PSUM") as psA,
        tc.tile_pool(name="psW", bufs=4, space="PSUM") as psW,
    ):
        xb_all = pp.tile([P, T * D], BF16)       # bf16 x, resident (4MB)
        nsq = pp.tile([P, T], BF16)              # row sum-of-squares
        nrm = pp.tile([P, T], F32)               # ||x_i||
        invn = pp.tile([P, T], F32)
        invn_bf = pp.tile([P, T], BF16)
        r_f = pp.tile([P, T], F32)               # x_i . t
        p_t = pp.tile([P, T], F32)
        sp_t = pp.tile([P, T], F32)
        f_t = pp.tile([P, T], F32)               # f = rsqrt(nrm * r)

        W_sb = pp.tile([P, 2 * D], F32)          # W k-chunk kc at [:, kc*D]
        Wb_sb = pp.tile([P, 2 * D], BF16)
        t_sb = pp.tile([1, D], BF16)
        ones_bf = pp.tile([1, P], BF16)
        tb_sb = pp.tile([P, D], BF16)            # t broadcast to 128 partitions
        ident_f = pp.tile([P, P], F32)
        ident_bf = pp.tile([P, P], BF16)
        xbT = pp.tile([P, 2 * NL], BF16)         # local x^T: block (i,c) at (2i+c)*P
        z_top_sb = pp.tile([P, D], BF16)         # [z11 | z12]
        z22_sb = pp.tile([P, P], BF16)
        z21_sb = pp.tile([P, P], BF16)
        zw_sb = pp.tile([P, 2 * D], BF16)        # zw rows 0:128 at [:,0:D], 128:256 at [:,D:2D]

        t_ps = psA.tile([1, D], F32, name="t_ps")
        tb_ps = psA.tile([P, D], F32, name="tb_ps")
        z_top_ps = psA.tile([P, D], F32, name="z_top_ps")
        z22_ps = psA.tile([P, P], F32, name="z22_ps")

        # x chunk DMAs first - everything else hides under them
        x_chs = []
        for c in range(NCH):
            x_ch = xp.tile([P, CW], F32, tag="xch", name=f"xch{c}")
            src = x[c * CH * P:(c + 1) * CH * P, :].rearrange(
                "(p j) d -> p j d", p=P
            )
            nc.sync.dma_start(x_ch[:].rearrange("p (j d) -> p j d", j=CH), src)
            x_chs.append(x_ch)

        for kc in range(2):
            nc.sync.dma_start(W_sb[:, kc * D:(kc + 1) * D], W[kc * P:(kc + 1) * P, :])

        masks.make_identity(nc, ident_f[:])
        nc.vector.tensor_copy(ident_bf[:], ident_f[:])
        nc.gpsimd.memset(ones_bf[:], 1.0)
        nc.scalar.copy(Wb_sb[:], W_sb[:])
        # preload both activation table sets during the DMA lead-in so the
        # ~1.3us ACT_TABLE_LOADs stay off the pass-1 critical chain
        warm = pp.tile([1, 8], F32)
        nc.scalar.activation(warm[:], ident_f[0:1, 0:8], AF.Square)
        nc.scalar.activation(warm[:], ident_f[0:1, 0:8], AF.Sqrt)

        # ---- pass 1 (overlapped with DMA): cast, row sumsq, t accumulation ----
        for c in range(NCH):
            x_ch = x_chs[c]
            cs = slice(c * CH, (c + 1) * CH)
            xb_ch = xb_all[:, c * CW:(c + 1) * CW]
            nc.vector.tensor_copy(xb_ch, x_ch[:])
            sq_ch = wp.tile([P, CW], BF16, tag="sq", name=f"sq{c}")
            nc.scalar.activation(sq_ch[:], x_ch[:], AF.Square)
            _rowsum(nc, nsq[:, cs], sq_ch[:].rearrange("p (t d) -> p t d", t=CH),
                    sq_ch[:].rearrange("p (t d) -> p t d", t=CH))
            nc.scalar.activation(nrm[:, cs], nsq[:, cs], AF.Sqrt)
            nc.vector.reciprocal(invn[:, cs], nrm[:, cs])
            nc.vector.tensor_copy(invn_bf[:, cs], invn[:, cs])

            if c == 0:
                # local x^T for the final GEMM - PE is idle during the load
                for i in range(TL):
                    for h in range(2):
                        pt = psW.tile([P, P], BF16, tag="pw", name=f"pt{i}_{h}")
                        nc.tensor.transpose(
                            pt[:], xb_all[:, i * D + h * P:i * D + (h + 1) * P],
                            ident_bf[:],
                        )
                        nc.scalar.copy(
                            xbT[:, (2 * i + h) * P:(2 * i + h + 1) * P], pt[:]
                        )

            for i in range(c * CH, (c + 1) * CH):
                nc.tensor.matmul(
                    t_ps[:], lhsT=invn_bf[:, i:i + 1],
                    rhs=xb_all[:, i * D:(i + 1) * D],
                    start=(i == 0), stop=(i == T - 1),
                )

        # ---- barrier: t ready; broadcast to 128 partitions ----
        nc.scalar.copy(t_sb[:], t_ps[:])
        nc.tensor.matmul(tb_ps[:], lhsT=ones_bf[:], rhs=t_sb[:], start=True, stop=True)
        nc.vector.tensor_copy(tb_sb[:], tb_ps[:])

        # ---- pass 2: r = x.t, f, g = f*x, z accumulation (PE) ----
        for c in range(NCH):
            cs = slice(c * CH, (c + 1) * CH)
            xb3 = xb_all[:, c * CW:(c + 1) * CW].rearrange("p (t d) -> p t d", t=CH)
            u_ch = wp.tile([P, D], BF16, tag="u", name=f"u{c}")
            # fused (xb * tb) + rowsum in one DVE op per tile; u_ch is a
            # rotating junk buffer for the mandatory full-tensor output
            for j in range(CH):
                i = c * CH + j
                nc.vector.scalar_tensor_tensor(
                    u_ch[:], xb_all[:, i * D:(i + 1) * D], 0.0, tb_sb[:],
                    op0=ALU.bypass, op1=ALU.mult,
                    accum_out=r_f[:, i:i + 1],
                )
            nc.vector.tensor_mul(p_t[:, cs], nrm[:, cs], r_f[:, cs])
            nc.scalar.activation(sp_t[:, cs], p_t[:, cs], AF.Sqrt)
            nc.vector.reciprocal(f_t[:, cs], sp_t[:, cs])

            g_ch = wp.tile([P, CW], BF16, tag="g", name=f"g{c}")
            for j in range(CH):
                i = c * CH + j
                g_i = g_ch[:, j * D:(j + 1) * D]
                if j % 2 == 0:
                    nc.scalar.mul(g_i, xb_all[:, i * D:(i + 1) * D],
                                  f_t[:, i:i + 1])
                else:
                    nc.vector.tensor_scalar_mul(g_i, xb_all[:, i * D:(i + 1) * D],
                                                f_t[:, i:i + 1])
                nc.tensor.matmul(
                    z_top_ps[:], lhsT=xb_all[:, i * D:i * D + P], rhs=g_i,
                    start=(i == 0), stop=(i == T - 1),
                )
                nc.tensor.matmul(
                    z22_ps[:], lhsT=xb_all[:, i * D + P:(i + 1) * D],
                    rhs=g_ch[:, j * D + P:(j + 1) * D],
                    start=(i == 0), stop=(i == T - 1),
                )

        # ---- zw = z @ W using symmetry (z21 = z12^T) ----
        nc.scalar.copy(z_top_sb[:], z_top_ps[:])
        nc.vector.tensor_copy(z22_sb[:], z22_ps[:])
        zT_ps = psW.tile([P, P], BF16, tag="pw", name="zT")
        nc.tensor.transpose(zT_ps[:], z_top_sb[:, P:D], ident_bf[:])
        nc.vector.tensor_copy(z21_sb[:], zT_ps[:])

        for half, (lhs1, lhs2) in enumerate(
            ((z_top_sb[:, 0:P], z21_sb[:]),        # zw_top = z11 W1 + (z12^T)^T W2
             (z_top_sb[:, P:D], z22_sb[:]))        # zw_bot = z12^T W1 + z22 W2
        ):
            zw_ps = psW.tile([P, D], F32, tag="pw", name=f"zw{half}")
            nc.tensor.matmul(zw_ps[:], lhsT=lhs1, rhs=Wb_sb[:, 0:D],
                             start=True, stop=False)
            nc.tensor.matmul(zw_ps[:], lhsT=lhs2, rhs=Wb_sb[:, D:2 * D],
                             start=False, stop=True)
            nc.vector.tensor_copy(zw_sb[:, half * D:(half + 1) * D], zw_ps[:])

        # ---- final: out_j = f_j * (x_j @ zw) for the 8 local tiles ----
        # scales land in one persistent tile; a single contiguous 1MB store
        # (8KB per partition) replaces 8 slow strided stores
        o_all = pp.tile([P, TL * D], F32)
        for i in range(TL):
            o_ps = psW.tile([P, D], F32, tag="pw", name=f"o{i}")
            for h in range(2):
                nc.tensor.matmul(
                    o_ps[:], lhsT=xbT[:, (2 * i + h) * P:(2 * i + h + 1) * P],
                    rhs=zw_sb[:, h * D:(h + 1) * D],
                    start=(h == 0), stop=(h == 1),
                )
            o_i = o_all[:, i * D:(i + 1) * D]
            if i % 2 == 0:
                nc.scalar.mul(o_i, o_ps[:], f_t[:, i:i + 1])
            else:
                nc.vector.tensor_scalar_mul(o_i, o_ps[:], f_t[:, i:i + 1])
        nc.sync.dma_start(
            out.rearrange("(p j) d -> p j d", p=P),
            o_all[:].rearrange("p (j d) -> p j d", j=TL),
        )


def _build():
    nc = bacc.Bacc("TRN2", target_bir_lowering=False, debug=False, num_devices=R)
    x = nc.dram_tensor("x", [N, D], F32, kind="ExternalInput")
    W = nc.dram_tensor("W", [D, D], F32, kind="ExternalInput")
    out = nc.dram_tensor("out", [NL, D], F32, kind="ExternalOutput")
    with nc.allow_low_precision("bf16 row reductions; validated ~1.6e-3 rel err"):
        with tile.TileContext(nc) as tc:
            _program(
                tc,
                x.ap() if hasattr(x, "ap") else x,
                W.ap() if hasattr(W, "ap") else W,
                out.ap() if hasattr(out, "ap") else out,
            )
    nc.finalize()
    return nc


def _run(inputs, trace=False):
    if "nc" not in _cache:
        _cache["nc"] = _build()
    nc = _cache["nc"]
    x = np.ascontiguousarray(inputs["x"], dtype=np.float32)
    W = np.ascontiguousarray(inputs["W"], dtype=np.float32)
    in_maps = [
        {"x": np.roll(x, -r * NL, axis=0), "W": W} for r in range(R)
    ]
    res = bass_utils.run_bass_kernel_spmd(
        nc, in_maps, core_ids=list(range(R)), trace=trace,
    )
    out = np.concatenate([res.results[r]["out"] for r in range(R)], axis=0)
    return out, res


def kernel(**inputs) -> np.ndarray:
    out, _ = _run(inputs, trace=False)
    return out


# revision 31
# speedup vs baseline: 1.2179x; 1.1420x over previous
"""Distributed Trainium2 kernel for the dense-graph GNN layer.

Math: with xn = x/||x|| (rows), G = xn@xn.T, d = rsqrt(G@1),
out = (diag(d) G diag(d) x) W.  The N x N Gram matrix is never needed:
  t = colsum(xn) = X^T invn                                   [D]
  r_i = x_i . t ;  f_i = rsqrt(||x_i|| * r_i)
  z = X^T diag(f) X                                           [D, D]
  out = f_loc * (X_loc @ (z @ W))

Distribution: collectives in this environment cost ~40us+ each (measured:
both AllGather and AllReduce stall every core for tens of us in the
axon/fake_nrt runtime), so this kernel uses ZERO collectives.  Every core
receives the FULL x (rolled so its own 1024 rows come first - the global
reductions t and z are permutation-invariant) and redundantly computes the
global reductions, then produces only its local 1024-row output slice.
Per-core cost is dominated by streaming the 8MB x from HBM once.

Row layout: within each 1024-row chunk, partition p holds rows 8p..8p+7
(8KB contiguous per partition per chunk -> efficient DMA descriptors).
All global reductions are row-permutation-invariant; the local output
store inverts the same mapping.
"""

import os
import sys

import numpy as np

for _p in ("/opt/trn_rl_repo", "/root/.axon_site/_ro/trn_rl_repo"):
    if os.path.isdir(_p) and _p not in sys.path:
        sys.path.insert(0, _p)

import concourse.bacc as bacc
import concourse.mybir as mybir
import concourse.tile as tile
import concourse.masks as masks
from concourse import bass_utils

R = 8                  # cores
N, D = 8192, 256
NL = N // R            # 1024 rows per core (local shard)
P = 128
T = N // P             # 64 row tiles per core (full x)
TL = NL // P           # 8 local row tiles
CH = 8                 # tiles per DMA chunk
NCH = T // CH          # 8 chunks
CW = CH * D            # chunk width in columns (2048)
F32 = mybir.dt.float32
BF16 = mybir.dt.bfloat16
AF = mybir.ActivationFunctionType
ALU = mybir.AluOpType
AX = mybir.AxisListType

_cache = {}


def _rowsum(nc, out_col, ch3, scratch3):
    """Row-sum of ch3 [128, CH, 256] -> out_col [128, CH].
    Three 2x halving adds into scratch, then a 1x reduce of [128, CH, 32]."""
    w = 128
    nc.vector.tensor_add(scratch3[:, :, 0:w], ch3[:, :, 0:w], ch3[:, :, w:2 * w])
    while w > 32:
        h = w // 2
        nc.vector.tensor_add(scratch3[:, :, 0:h], scratch3[:, :, 0:h],
                             scratch3[:, :, h:2 * h])
        w = h
    nc.vector.tensor_reduce(out_col, scratch3[:, :, 0:w], axis=AX.X, op=ALU.add)


def _program(tc, x, W, out):
    nc = tc.nc
    with (
        tc.tile_pool(name="persist", bufs=1) as pp,
        tc.tile_pool(name="xin", bufs=3) as xp,
        tc.tile_pool(name="work", bufs=3) as wp,
        tc.tile_pool(name="psA", bufs=1, space="PSUM") as psA,
        tc.tile_pool(name="psW", bufs=4, space="PSUM") as psW,
    ):
        xb_all = pp.tile([P, T * D], BF16)       # bf16 x, resident (4MB)
        nsq = pp.tile([P, T], BF16)              # row sum-of-squares
        nrm = pp.tile([P, T], F32)               # ||x_i||
        invn = pp.tile([P, T], F32)
        invn_bf = pp.tile([P, T], BF16)
        r_f = pp.tile([P, T], F32)               # x_i . t
        p_t = pp.tile([P, T], F32)
        sp_t = pp.tile([P, T], F32)
        f_t = pp.tile([P, T], F32)               # f = rsqrt(nrm * r)

        W_sb = pp.tile([P, 2 * D], F32)          # W k-chunk kc at [:, kc*D]
        Wb_sb = pp.tile([P, 2 * D], BF16)
        t_sb = pp.tile([1, D], BF16)
        ones_bf = pp.tile([1, P], BF16)
        tb_sb = pp.tile([P, D], BF16)            # t broadcast to 128 partitions
        ident_f = pp.tile([P, P], F32)
        ident_bf = pp.tile([P, P], BF16)
        xbT = pp.tile([P, 2 * NL], BF16)         # local x^T: block (i,c) at (2i+c)*P
        z_top_sb = pp.tile([P, D], BF16)         # [z11 | z12]
        z22_sb = pp.tile([P, P], BF16)
        z21_sb = pp.tile([P, P], BF16)
        zw_sb = pp.tile([P, 2 * D], BF16)        # zw rows 0:128 at [:,0:D], 128:256 at [:,D:2D]

        t_ps = psA.tile([1, D], F32, name="t_ps")
        tb_ps = psA.tile([P, D], F32, name="tb_ps")
        z_top_ps = psA.tile([P, D], F32, name="z_top_ps")
        z22_ps = psA.tile([P, P], F32, name="z22_ps")

        # x chunk DMAs first - everything else hides under them
        x_chs = []
        for c in range(NCH):
            x_ch = xp.tile([P, CW], F32, tag="xch", name=f"xch{c}")
            src = x[c * CH * P:(c + 1) * CH * P, :].rearrange(
                "(p j) d -> p j d", p=P
            )
            nc.sync.dma_start(x_ch[:].rearrange("p (j d) -> p j d", j=CH), src)
            x_chs.append(x_ch)

        for kc in range(2):
            nc.sync.dma_start(W_sb[:, kc * D:(kc + 1) * D], W[kc * P:(kc + 1) * P, :])

        masks.make_identity(nc, ident_f[:])
        nc.vector.tensor_copy(ident_bf[:], ident_f[:])
        nc.gpsimd.memset(ones_bf[:], 1.0)
        nc.scalar.copy(Wb_sb[:], W_sb[:])
        # preload both activation table sets during the DMA lead-in so the
        # ~1.3us ACT_TABLE_LOADs stay off the pass-1 critical chain
        warm = pp.tile([1, 8], F32)
        nc.scalar.activation(warm[:], ident_f[0:1, 0:8], AF.Square)
        nc.scalar.activation(warm[:], ident_f[0:1, 0:8], AF.Sqrt)

        # ---- pass 1 (overlapped with DMA): cast, row sumsq, t accumulation ----
        for c in range(NCH):
            x_ch = x_chs[c]
            cs = slice(c * CH, (c + 1) * CH)
            xb_ch = xb_all[:, c * CW:(c + 1) * CW]
            nc.vector.tensor_copy(xb_ch, x_ch[:])
            sq_ch = wp.tile([P, CW], BF16, tag="sq", name=f"sq{c}")
            nc.scalar.activation(sq_ch[:], x_ch[:], AF.Square)
            _rowsum(nc, nsq[:, cs], sq_ch[:].rearrange("p (t d) -> p t d", t=CH),
                    sq_ch[:].rearrange("p (t d) -> p t d", t=CH))
            nc.scalar.activation(nrm[:, cs], nsq[:, cs], AF.Sqrt)
            nc.vector.reciprocal(invn[:, cs], nrm[:, cs])
            nc.vector.tensor_copy(invn_bf[:, cs], invn[:, cs])

            if c == 0:
                # local x^T for the final GEMM - PE is idle during the load
                for i in range(TL):
                    for h in range(2):
                        pt = psW.tile([P, P], BF16, tag="pw", name=f"pt{i}_{h}")
                        nc.tensor.transpose(
                            pt[:], xb_all[:, i * D + h * P:i * D + (h + 1) * P],
                            ident_bf[:],
                        )
                        nc.scalar.copy(
                            xbT[:, (2 * i + h) * P:(2 * i + h + 1) * P], pt[:]
                        )

            for i in range(c * CH, (c + 1) * CH):
                nc.tensor.matmul(
                    t_ps[:], lhsT=invn_bf[:, i:i + 1],
                    rhs=xb_all[:, i * D:(i + 1) * D],
                    start=(i == 0), stop=(i == T - 1),
                )

        # ---- barrier: t ready; broadcast to 128 partitions ----
        nc.scalar.copy(t_sb[:], t_ps[:])
        nc.tensor.matmul(tb_ps[:], lhsT=ones_bf[:], rhs=t_sb[:], start=True, stop=True)
        nc.vector.tensor_copy(tb_sb[:], tb_ps[:])

        # ---- pass 2: r = x.t, f, g = f*x, z accumulation (PE) ----
        for c in range(NCH):
            cs = slice(c * CH, (c + 1) * CH)
            xb3 = xb_all[:, c * CW:(c + 1) * CW].rearrange("p (t d) -> p t d", t=CH)
            u_ch = wp.tile([P, D], BF16, tag="u", name=f"u{c}")
            # fused (xb * tb) + rowsum in one DVE op per tile; u_ch is a
            # rotating junk buffer for the mandatory full-tensor output
            for j in range(CH):
                i = c * CH + j
                nc.vector.scalar_tensor_tensor(
                    u_ch[:], xb_all[:, i * D:(i + 1) * D], 0.0, tb_sb[:],
                    op0=ALU.bypass, op1=ALU.mult,
                    accum_out=r_f[:, i:i + 1],
                )
            nc.vector.tensor_mul(p_t[:, cs], nrm[:, cs], r_f[:, cs])
            nc.scalar.activation(sp_t[:, cs], p_t[:, cs], AF.Sqrt)
            nc.vector.reciprocal(f_t[:, cs], sp_t[:, cs])

            g_ch = wp.tile([P, CW], BF16, tag="g", name=f"g{c}")
            for j in range(CH):
                i = c * CH + j
                g_i = g_ch[:, j * D:(j + 1) * D]
                if j % 2 == 0:
                    nc.scalar.mul(g_i, xb_all[:, i * D:(i + 1) * D],
                                  f_t[:, i:i + 1])
                else:
                    nc.vector.tensor_scalar_mul(g_i, xb_all[:, i * D:(i + 1) * D],
                                                f_t[:, i:i + 1])
                nc.tensor.matmul(
                    z_top_ps[:], lhsT=xb_all[:, i * D:i * D + P], rhs=g_i,
                    start=(i == 0), stop=(i == T - 1),
                )
                nc.tensor.matmul(
                    z22_ps[:], lhsT=xb_all[:, i * D + P:(i + 1) * D],
                    rhs=g_ch[:, j * D + P:(j + 1) * D],
                    start=(i == 0), stop=(i == T - 1),
                )

        # ---- zw = z @ W using symmetry (z21 = z12^T) ----
        nc.scalar.copy(z_top_sb[:], z_top_ps[:])
        nc.vector.tensor_copy(z22_sb[:], z22_ps[:])
        zT_ps = psW.tile([P, P], BF16, tag="pw", name="zT")
        nc.tensor.transpose(zT_ps[:], z_top_sb[:, P:D], ident_bf[:])
        nc.vector.tensor_copy(z21_sb[:], zT_ps[:])

        for half, (lhs1, lhs2) in enumerate(
            ((z_top_sb[:, 0:P], z21_sb[:]),        # zw_top = z11 W1 + (z12^T)^T W2
             (z_top_sb[:, P:D], z22_sb[:]))        # zw_bot = z12^T W1 + z22 W2
        ):
            zw_ps = psW.tile([P, D], F32, tag="pw", name=f"zw{half}")
            nc.tensor.matmul(zw_ps[:], lhsT=lhs1, rhs=Wb_sb[:, 0:D],
                             start=True, stop=False)
            nc.tensor.matmul(zw_ps[:], lhsT=lhs2, rhs=Wb_sb[:, D:2 * D],
                             start=False, stop=True)
            nc.vector.tensor_copy(zw_sb[:, half * D:(half + 1) * D], zw_ps[:])

        # ---- final: out_j = f_j * (x_j @ zw) for the 8 local tiles ----
        # scales land in one persistent tile (no buffer-rotation gating);
        # 8 per-tile stores run on parallel DMA queues
        out3 = out.rearrange("(p j) d -> p j d", p=P)
        o_all = pp.tile([P, TL * D], F32)
        for i in range(TL):
            o_ps = psW.tile([P, D], F32, tag="pw", name=f"o{i}")
            for h in range(2):
                nc.tensor.matmul(
                    o_ps[:], lhsT=xbT[:, (2 * i + h) * P:(2 * i + h + 1) * P],
                    rhs=zw_sb[:, h * D:(h + 1) * D],
                    start=(h == 0), stop=(h == 1),
                )
            o_i = o_all[:, i * D:(i + 1) * D]
            if i % 2 == 0:
                nc.scalar.mul(o_i, o_ps[:], f_t[:, i:i + 1])
            else:
                nc.vector.tensor_scalar_mul(o_i, o_ps[:], f_t[:, i:i + 1])
            nc.sync.dma_start(out3[:, i, :], o_i)


def _build():
    nc = bacc.Bacc("TRN2", target_bir_lowering=False, debug=False, num_devices=R)
    x = nc.dram_tensor("x", [N, D], F32, kind="ExternalInput")
    W = nc.dram_tensor("W", [D, D], F32, kind="ExternalInput")
    out = nc.dram_tensor("out", [NL, D], F32, kind="ExternalOutput")
    with nc.allow_low_precision("bf16 row reductions; validated ~1.6e-3 rel err"):
        with tile.TileContext(nc) as tc:
            _program(
                tc,
                x.ap() if hasattr(x, "ap") else x,
                W.ap() if hasattr(W, "ap") else W,
                out.ap() if hasattr(out, "ap") else out,
            )
    nc.finalize()
    return nc


def _run(inputs, trace=False):
    if "nc" not in _cache:
        _cache["nc"] = _build()
    nc = _cache["nc"]
    x = np.ascontiguousarray(inputs["x"], dtype=np.float32)
    W = np.ascontiguousarray(inputs["W"], dtype=np.float32)
    in_maps = [
        {"x": np.roll(x, -r * NL, axis=0), "W": W} for r in range(R)
    ]
    res = bass_utils.run_bass_kernel_spmd(
        nc, in_maps, core_ids=list(range(R)), trace=trace,
    )
    out = np.concatenate([res.results[r]["out"] for r in range(R)], axis=0)
    return out, res


def kernel(**inputs) -> np.ndarray:
    out, _ = _run(inputs, trace=False)
    return out


# revision 33
# speedup vs baseline: 1.2525x; 1.0284x over previous
"""Distributed Trainium2 kernel for the dense-graph GNN layer.

Math: with xn = x/||x|| (rows), G = xn@xn.T, d = rsqrt(G@1),
out = (diag(d) G diag(d) x) W.  The N x N Gram matrix is never needed:
  t = colsum(xn) = X^T invn                                   [D]
  r_i = x_i . t ;  f_i = rsqrt(||x_i|| * r_i)
  z = X^T diag(f) X                                           [D, D]
  out = f_loc * (X_loc @ (z @ W))

Distribution: collectives in this environment cost ~40us+ each (measured:
both AllGather and AllReduce stall every core for tens of us in the
axon/fake_nrt runtime), so this kernel uses ZERO collectives.  Every core
receives the FULL x (rolled so its own 1024 rows come first - the global
reductions t and z are permutation-invariant) and redundantly computes the
global reductions, then produces only its local 1024-row output slice.
Per-core cost is dominated by streaming the 8MB x from HBM once.

Row layout: within each 1024-row chunk, partition p holds rows 8p..8p+7
(8KB contiguous per partition per chunk -> efficient DMA descriptors).
All global reductions are row-permutation-invariant; the local output
store inverts the same mapping.
"""

import os
import sys

import numpy as np

for _p in ("/opt/trn_rl_repo", "/root/.axon_site/_ro/trn_rl_repo"):
    if os.path.isdir(_p) and _p not in sys.path:
        sys.path.insert(0, _p)

import concourse.bacc as bacc
import concourse.mybir as mybir
import concourse.tile as tile
import concourse.masks as masks
from concourse import bass_utils

R = 8                  # cores
N, D = 8192, 256
NL = N // R            # 1024 rows per core (local shard)
P = 128
T = N // P             # 64 row tiles per core (full x)
TL = NL // P           # 8 local row tiles
CH = 8                 # tiles per DMA chunk
NCH = T // CH          # 8 chunks
CW = CH * D            # chunk width in columns (2048)
F32 = mybir.dt.float32
BF16 = mybir.dt.bfloat16
AF = mybir.ActivationFunctionType
ALU = mybir.AluOpType
AX = mybir.AxisListType

_cache = {}


def _rowsum(nc, out_col, ch3, scratch3):
    """Row-sum of ch3 [128, CH, 256] -> out_col [128, CH].
    Three 2x halving adds into scratch, then a 1x reduce of [128, CH, 32]."""
    w = 128
    nc.vector.tensor_add(scratch3[:, :, 0:w], ch3[:, :, 0:w], ch3[:, :, w:2 * w])
    while w > 32:
        h = w // 2
        nc.vector.tensor_add(scratch3[:, :, 0:h], scratch3[:, :, 0:h],
                             scratch3[:, :, h:2 * h])
        w = h
    nc.vector.tensor_reduce(out_col, scratch3[:, :, 0:w], axis=AX.X, op=ALU.add)


def _program(tc, x, W, out):
    nc = tc.nc
    with (
        tc.tile_pool(name="persist", bufs=1) as pp,
        tc.tile_pool(name="xin", bufs=3) as xp,
        tc.tile_pool(name="work", bufs=3) as wp,
        tc.tile_pool(name="psA", bufs=1, space="PSUM") as psA,
        tc.tile_pool(name="psW", bufs=4, space="PSUM") as psW,
    ):
        xb_all = pp.tile([P, T * D], BF16)       # bf16 x, resident (4MB)
        nsq = pp.tile([P, T], BF16)              # row sum-of-squares
        nrm = pp.tile([P, T], F32)               # ||x_i||
        invn = pp.tile([P, T], F32)
        invn_bf = pp.tile([P, T], BF16)
        r_f = pp.tile([P, T], F32)               # x_i . t
        p_t = pp.tile([P, T], F32)
        sp_t = pp.tile([P, T], F32)
        f_t = pp.tile([P, T], F32)               # f = rsqrt(nrm * r)

        W_sb = pp.tile([P, 2 * D], F32)          # W k-chunk kc at [:, kc*D]
        Wb_sb = pp.tile([P, 2 * D], BF16)
        t_sb = pp.tile([1, D], BF16)
        ones_bf = pp.tile([1, P], BF16)
        tb_sb = pp.tile([P, D], BF16)            # t broadcast to 128 partitions
        ident_f = pp.tile([P, P], F32)
        ident_bf = pp.tile([P, P], BF16)
        xbT = pp.tile([P, 2 * NL], BF16)         # local x^T: block (i,c) at (2i+c)*P
        z_top_sb = pp.tile([P, D], BF16)         # [z11 | z12]
        z22_sb = pp.tile([P, P], BF16)
        z21_sb = pp.tile([P, P], BF16)
        zw_sb = pp.tile([P, 2 * D], BF16)        # zw rows 0:128 at [:,0:D], 128:256 at [:,D:2D]

        t_ps = psA.tile([1, D], F32, name="t_ps")
        tb_ps = psA.tile([P, D], F32, name="tb_ps")
        z_top_ps = psA.tile([P, D], F32, name="z_top_ps")
        z22_ps = psA.tile([P, P], F32, name="z22_ps")

        # x chunk DMAs first - everything else hides under them
        x_chs = []
        for c in range(NCH):
            x_ch = xp.tile([P, CW], F32, tag="xch", name=f"xch{c}")
            src = x[c * CH * P:(c + 1) * CH * P, :].rearrange(
                "(p j) d -> p j d", p=P
            )
            nc.sync.dma_start(x_ch[:].rearrange("p (j d) -> p j d", j=CH), src)
            x_chs.append(x_ch)

        for kc in range(2):
            nc.sync.dma_start(W_sb[:, kc * D:(kc + 1) * D], W[kc * P:(kc + 1) * P, :])

        masks.make_identity(nc, ident_f[:])
        nc.vector.tensor_copy(ident_bf[:], ident_f[:])
        nc.gpsimd.memset(ones_bf[:], 1.0)
        nc.scalar.copy(Wb_sb[:], W_sb[:])
        # preload both activation table sets during the DMA lead-in so the
        # ~1.3us ACT_TABLE_LOADs stay off the pass-1 critical chain
        warm = pp.tile([1, 8], F32)
        nc.scalar.activation(warm[:], ident_f[0:1, 0:8], AF.Square)
        nc.scalar.activation(warm[:], ident_f[0:1, 0:8], AF.Sqrt)

        # ---- pass 1 (overlapped with DMA): cast, row sumsq, t accumulation ----
        for c in range(NCH):
            x_ch = x_chs[c]
            cs = slice(c * CH, (c + 1) * CH)
            xb_ch = xb_all[:, c * CW:(c + 1) * CW]
            nc.vector.tensor_copy(xb_ch, x_ch[:])
            sq_ch = wp.tile([P, CW], BF16, tag="sq", name=f"sq{c}")
            nc.scalar.activation(sq_ch[:], x_ch[:], AF.Square)
            _rowsum(nc, nsq[:, cs], sq_ch[:].rearrange("p (t d) -> p t d", t=CH),
                    sq_ch[:].rearrange("p (t d) -> p t d", t=CH))
            nc.scalar.activation(nrm[:, cs], nsq[:, cs], AF.Sqrt)
            nc.vector.reciprocal(invn[:, cs], nrm[:, cs])
            nc.vector.tensor_copy(invn_bf[:, cs], invn[:, cs])

            if c == 0:
                # local x^T for the final GEMM - PE is idle during the load
                for i in range(TL):
                    for h in range(2):
                        pt = psW.tile([P, P], BF16, tag="pw", name=f"pt{i}_{h}")
                        nc.tensor.transpose(
                            pt[:], xb_all[:, i * D + h * P:i * D + (h + 1) * P],
                            ident_bf[:],
                        )
                        nc.scalar.copy(
                            xbT[:, (2 * i + h) * P:(2 * i + h + 1) * P], pt[:]
                        )

            for i in range(c * CH, (c + 1) * CH):
                nc.tensor.matmul(
                    t_ps[:], lhsT=invn_bf[:, i:i + 1],
                    rhs=xb_all[:, i * D:(i + 1) * D],
                    start=(i == 0), stop=(i == T - 1),
                )

        # ---- barrier: t ready; broadcast to 128 partitions ----
        nc.scalar.copy(t_sb[:], t_ps[:])
        nc.tensor.matmul(tb_ps[:], lhsT=ones_bf[:], rhs=t_sb[:], start=True, stop=True)
        nc.vector.tensor_copy(tb_sb[:], tb_ps[:])

        # ---- pass 2: r = x.t, f, g = f*x, z accumulation (PE) ----
        for c in range(NCH):
            cs = slice(c * CH, (c + 1) * CH)
            xb3 = xb_all[:, c * CW:(c + 1) * CW].rearrange("p (t d) -> p t d", t=CH)
            u_ch = wp.tile([P, D], BF16, tag="u", name=f"u{c}")
            # fused (xb * tb) + rowsum in one DVE op per tile; u_ch is a
            # rotating junk buffer for the mandatory full-tensor output
            for j in range(CH):
                i = c * CH + j
                nc.vector.scalar_tensor_tensor(
                    u_ch[:], xb_all[:, i * D:(i + 1) * D], 0.0, tb_sb[:],
                    op0=ALU.bypass, op1=ALU.mult,
                    accum_out=r_f[:, i:i + 1],
                )
            nc.vector.tensor_mul(p_t[:, cs], nrm[:, cs], r_f[:, cs])
            nc.scalar.activation(sp_t[:, cs], p_t[:, cs], AF.Sqrt)
            nc.vector.reciprocal(f_t[:, cs], sp_t[:, cs])

            g_ch = wp.tile([P, CW], BF16, tag="g", name=f"g{c}")
            for j in range(CH):
                i = c * CH + j
                g_i = g_ch[:, j * D:(j + 1) * D]
                if j % 8 in (0, 2, 4, 5, 7):
                    nc.scalar.mul(g_i, xb_all[:, i * D:(i + 1) * D],
                                  f_t[:, i:i + 1])
                else:
                    nc.vector.tensor_scalar_mul(g_i, xb_all[:, i * D:(i + 1) * D],
                                                f_t[:, i:i + 1])
                nc.tensor.matmul(
                    z_top_ps[:], lhsT=xb_all[:, i * D:i * D + P], rhs=g_i,
                    start=(i == 0), stop=(i == T - 1),
                )
                nc.tensor.matmul(
                    z22_ps[:], lhsT=xb_all[:, i * D + P:(i + 1) * D],
                    rhs=g_ch[:, j * D + P:(j + 1) * D],
                    start=(i == 0), stop=(i == T - 1),
                )

        # ---- zw = z @ W using symmetry (z21 = z12^T) ----
        nc.scalar.copy(z_top_sb[:], z_top_ps[:])
        nc.vector.tensor_copy(z22_sb[:], z22_ps[:])
        zT_ps = psW.tile([P, P], BF16, tag="pw", name="zT")
        nc.tensor.transpose(zT_ps[:], z_top_sb[:, P:D], ident_bf[:])
        nc.vector.tensor_copy(z21_sb[:], zT_ps[:])

        for half, (lhs1, lhs2) in enumerate(
            ((z_top_sb[:, 0:P], z21_sb[:]),        # zw_top = z11 W1 + (z12^T)^T W2
             (z_top_sb[:, P:D], z22_sb[:]))        # zw_bot = z12^T W1 + z22 W2
        ):
            zw_ps = psW.tile([P, D], F32, tag="pw", name=f"zw{half}")
            nc.tensor.matmul(zw_ps[:], lhsT=lhs1, rhs=Wb_sb[:, 0:D],
                             start=True, stop=False)
            nc.tensor.matmul(zw_ps[:], lhsT=lhs2, rhs=Wb_sb[:, D:2 * D],
                             start=False, stop=True)
            nc.vector.tensor_copy(zw_sb[:, half * D:(half + 1) * D], zw_ps[:])

        # ---- final: out_j = f_j * (x_j @ zw) for the 8 local tiles ----
        # scales land in one persistent tile (no buffer-rotation gating);
        # 8 per-tile stores run on parallel DMA queues
        out3 = out.rearrange("(p j) d -> p j d", p=P)
        o_all = pp.tile([P, TL * D], F32)
        for i in range(TL):
            o_ps = psW.tile([P, D], F32, tag="pw", name=f"o{i}")
            for h in range(2):
                nc.tensor.matmul(
                    o_ps[:], lhsT=xbT[:, (2 * i + h) * P:(2 * i + h + 1) * P],
                    rhs=zw_sb[:, h * D:(h + 1) * D],
                    start=(h == 0), stop=(h == 1),
                )
            o_i = o_all[:, i * D:(i + 1) * D]
            if i % 2 == 0:
                nc.scalar.mul(o_i, o_ps[:], f_t[:, i:i + 1])
            else:
                nc.vector.tensor_scalar_mul(o_i, o_ps[:], f_t[:, i:i + 1])
            if i % 2 == 1:
                nc.sync.dma_start(
                    out3[:, i - 1:i + 1, :],
                    o_all[:, (i - 1) * D:(i + 1) * D].rearrange(
                        "p (j d) -> p j d", j=2),
                )


def _build():
    nc = bacc.Bacc("TRN2", target_bir_lowering=False, debug=False, num_devices=R)
    x = nc.dram_tensor("x", [N, D], F32, kind="ExternalInput")
    W = nc.dram_tensor("W", [D, D], F32, kind="ExternalInput")
    out = nc.dram_tensor("out", [NL, D], F32, kind="ExternalOutput")
    with nc.allow_low_precision("bf16 row reductions; validated ~1.6e-3 rel err"):
        with tile.TileContext(nc) as tc:
            _program(
                tc,
                x.ap() if hasattr(x, "ap") else x,
                W.ap() if hasattr(W, "ap") else W,
                out.ap() if hasattr(out, "ap") else out,
            )
    nc.finalize()
    return nc


def _run(inputs, trace=False):
    if "nc" not in _cache:
        _cache["nc"] = _build()
    nc = _cache["nc"]
    x = np.ascontiguousarray(inputs["x"], dtype=np.float32)
    W = np.ascontiguousarray(inputs["W"], dtype=np.float32)
    in_maps = [
        {"x": np.roll(x, -r * NL, axis=0), "W": W} for r in range(R)
    ]
    res = bass_utils.run_bass_kernel_spmd(
        nc, in_maps, core_ids=list(range(R)), trace=trace,
    )
    out = np.concatenate([res.results[r]["out"] for r in range(R)], axis=0)
    return out, res


def kernel(**inputs) -> np.ndarray:
    out, _ = _run(inputs, trace=False)
    return out


# revision 35
# speedup vs baseline: 1.2912x; 1.0309x over previous
"""Distributed Trainium2 kernel for the dense-graph GNN layer.

Math: with xn = x/||x|| (rows), G = xn@xn.T, d = rsqrt(G@1),
out = (diag(d) G diag(d) x) W.  The N x N Gram matrix is never needed:
  t = colsum(xn) = X^T invn                                   [D]
  r_i = x_i . t ;  f_i = rsqrt(||x_i|| * r_i)
  z = X^T diag(f) X                                           [D, D]
  out = f_loc * (X_loc @ (z @ W))

Distribution: collectives in this environment cost ~40us+ each (measured:
both AllGather and AllReduce stall every core for tens of us in the
axon/fake_nrt runtime), so this kernel uses ZERO collectives.  Every core
receives the FULL x (rolled so its own 1024 rows come first - the global
reductions t and z are permutation-invariant) and redundantly computes the
global reductions, then produces only its local 1024-row output slice.
Per-core cost is dominated by streaming the 8MB x from HBM once.

Row layout: within each 1024-row chunk, partition p holds rows 8p..8p+7
(8KB contiguous per partition per chunk -> efficient DMA descriptors).
All global reductions are row-permutation-invariant; the local output
store inverts the same mapping.
"""

import os
import sys

import numpy as np

for _p in ("/opt/trn_rl_repo", "/root/.axon_site/_ro/trn_rl_repo"):
    if os.path.isdir(_p) and _p not in sys.path:
        sys.path.insert(0, _p)

import concourse.bacc as bacc
import concourse.mybir as mybir
import concourse.tile as tile
import concourse.masks as masks
from concourse import bass_utils

R = 8                  # cores
N, D = 8192, 256
NL = N // R            # 1024 rows per core (local shard)
P = 128
T = N // P             # 64 row tiles per core (full x)
TL = NL // P           # 8 local row tiles
CH = 8                 # tiles per DMA chunk
NCH = T // CH          # 8 chunks
CW = CH * D            # chunk width in columns (2048)
F32 = mybir.dt.float32
BF16 = mybir.dt.bfloat16
AF = mybir.ActivationFunctionType
ALU = mybir.AluOpType
AX = mybir.AxisListType

_cache = {}


def _rowsum(nc, out_col, ch3, scratch3):
    """Row-sum of ch3 [128, CH, 256] -> out_col [128, CH].
    Three 2x halving adds into scratch, then a 1x reduce of [128, CH, 32]."""
    w = 128
    nc.vector.tensor_add(scratch3[:, :, 0:w], ch3[:, :, 0:w], ch3[:, :, w:2 * w])
    while w > 32:
        h = w // 2
        nc.vector.tensor_add(scratch3[:, :, 0:h], scratch3[:, :, 0:h],
                             scratch3[:, :, h:2 * h])
        w = h
    nc.vector.tensor_reduce(out_col, scratch3[:, :, 0:w], axis=AX.X, op=ALU.add)


def _program(tc, x, W, out):
    nc = tc.nc
    with (
        tc.tile_pool(name="persist", bufs=1) as pp,
        tc.tile_pool(name="xin", bufs=3) as xp,
        tc.tile_pool(name="work", bufs=3) as wp,
        tc.tile_pool(name="psA", bufs=1, space="PSUM") as psA,
        tc.tile_pool(name="psW", bufs=4, space="PSUM") as psW,
    ):
        xb_all = pp.tile([P, T * D], BF16)       # bf16 x, resident (4MB)
        nsq = pp.tile([P, T], BF16)              # row sum-of-squares
        nrm = pp.tile([P, T], F32)               # ||x_i||
        invn = pp.tile([P, T], F32)
        invn_bf = pp.tile([P, T], BF16)
        r_f = pp.tile([P, T], F32)               # x_i . t
        p_t = pp.tile([P, T], F32)
        sp_t = pp.tile([P, T], F32)
        f_t = pp.tile([P, T], F32)               # f = rsqrt(nrm * r)

        W_sb = pp.tile([P, 2 * D], F32)          # W k-chunk kc at [:, kc*D]
        Wb_sb = pp.tile([P, 2 * D], BF16)
        t_sb = pp.tile([1, D], BF16)
        ones_bf = pp.tile([1, P], BF16)
        tb_sb = pp.tile([P, D], BF16)            # t broadcast to 128 partitions
        ident_f = pp.tile([P, P], F32)
        ident_bf = pp.tile([P, P], BF16)
        xbT = pp.tile([P, 2 * NL], BF16)         # local x^T: block (i,c) at (2i+c)*P
        z_top_sb = pp.tile([P, D], BF16)         # [z11 | z12]
        z22_sb = pp.tile([P, P], BF16)
        z21_sb = pp.tile([P, P], BF16)
        zw_sb = pp.tile([P, 2 * D], BF16)        # zw rows 0:128 at [:,0:D], 128:256 at [:,D:2D]

        t_ps = psA.tile([1, D], F32, name="t_ps")
        tb_ps = psA.tile([P, D], F32, name="tb_ps")
        z_top_ps = psA.tile([P, D], F32, name="z_top_ps")
        z22_ps = psA.tile([P, P], F32, name="z22_ps")

        # x chunk DMAs first - everything else hides under them
        x_chs = []
        for c in range(NCH):
            x_ch = xp.tile([P, CW], F32, tag="xch", name=f"xch{c}")
            src = x[c * CH * P:(c + 1) * CH * P, :].rearrange(
                "(p j) d -> p j d", p=P
            )
            nc.sync.dma_start(x_ch[:].rearrange("p (j d) -> p j d", j=CH), src)
            x_chs.append(x_ch)

        for kc in range(2):
            nc.sync.dma_start(W_sb[:, kc * D:(kc + 1) * D], W[kc * P:(kc + 1) * P, :])

        masks.make_identity(nc, ident_f[:])
        nc.vector.tensor_copy(ident_bf[:], ident_f[:])
        nc.gpsimd.memset(ones_bf[:], 1.0)
        nc.scalar.copy(Wb_sb[:], W_sb[:])
        # preload both activation table sets during the DMA lead-in so the
        # ~1.3us ACT_TABLE_LOADs stay off the pass-1 critical chain
        warm = pp.tile([1, 8], F32)
        nc.scalar.activation(warm[:], ident_f[0:1, 0:8], AF.Square)
        nc.scalar.activation(warm[:], ident_f[0:1, 0:8], AF.Sqrt)

        # ---- pass 1 (overlapped with DMA): cast, row sumsq, t accumulation ----
        for c in range(NCH):
            x_ch = x_chs[c]
            cs = slice(c * CH, (c + 1) * CH)
            xb_ch = xb_all[:, c * CW:(c + 1) * CW]
            nc.vector.tensor_copy(xb_ch, x_ch[:])
            sq_ch = wp.tile([P, CW], BF16, tag="sq", name=f"sq{c}")
            nc.scalar.activation(sq_ch[:], x_ch[:], AF.Square)
            _rowsum(nc, nsq[:, cs], sq_ch[:].rearrange("p (t d) -> p t d", t=CH),
                    sq_ch[:].rearrange("p (t d) -> p t d", t=CH))
            nc.scalar.activation(nrm[:, cs], nsq[:, cs], AF.Sqrt)
            nc.vector.reciprocal(invn[:, cs], nrm[:, cs])
            nc.vector.tensor_copy(invn_bf[:, cs], invn[:, cs])

            if c == 0:
                # local x^T for the final GEMM - PE is idle during the load
                for i in range(TL):
                    for h in range(2):
                        pt = psW.tile([P, P], BF16, tag="pw", name=f"pt{i}_{h}")
                        nc.tensor.transpose(
                            pt[:], xb_all[:, i * D + h * P:i * D + (h + 1) * P],
                            ident_bf[:],
                        )
                        nc.scalar.copy(
                            xbT[:, (2 * i + h) * P:(2 * i + h + 1) * P], pt[:]
                        )

            for i in range(c * CH, (c + 1) * CH):
                nc.tensor.matmul(
                    t_ps[:], lhsT=invn_bf[:, i:i + 1],
                    rhs=xb_all[:, i * D:(i + 1) * D],
                    start=(i == 0), stop=(i == T - 1),
                )

        # ---- barrier: t ready; broadcast to 128 partitions ----
        nc.scalar.copy(t_sb[:], t_ps[:])
        nc.tensor.matmul(tb_ps[:], lhsT=ones_bf[:], rhs=t_sb[:], start=True, stop=True)
        nc.vector.tensor_copy(tb_sb[:], tb_ps[:])

        # ---- pass 2: r = x.t, f, g = f*x, z accumulation (PE) ----
        for c in range(NCH):
            cs = slice(c * CH, (c + 1) * CH)
            xb3 = xb_all[:, c * CW:(c + 1) * CW].rearrange("p (t d) -> p t d", t=CH)
            u_ch = wp.tile([P, D], BF16, tag="u", name=f"u{c}")
            # fused (xb * tb) + rowsum in one DVE op per tile; u_ch is a
            # rotating junk buffer for the mandatory full-tensor output
            for j in range(CH):
                i = c * CH + j
                nc.vector.scalar_tensor_tensor(
                    u_ch[:], xb_all[:, i * D:(i + 1) * D], 0.0, tb_sb[:],
                    op0=ALU.bypass, op1=ALU.mult,
                    accum_out=r_f[:, i:i + 1],
                )
            nc.vector.tensor_mul(p_t[:, cs], nrm[:, cs], r_f[:, cs])
            nc.scalar.activation(sp_t[:, cs], p_t[:, cs], AF.Sqrt)
            nc.vector.reciprocal(f_t[:, cs], sp_t[:, cs])

            g_ch = wp.tile([P, CW], BF16, tag="g", name=f"g{c}")
            for j in range(CH):
                i = c * CH + j
                g_i = g_ch[:, j * D:(j + 1) * D]
                if j % 8 in (0, 2, 4, 5, 7):
                    nc.scalar.mul(g_i, xb_all[:, i * D:(i + 1) * D],
                                  f_t[:, i:i + 1])
                else:
                    nc.vector.tensor_scalar_mul(g_i, xb_all[:, i * D:(i + 1) * D],
                                                f_t[:, i:i + 1])
                nc.tensor.matmul(
                    z_top_ps[:], lhsT=xb_all[:, i * D:i * D + P], rhs=g_i,
                    start=(i == 0), stop=(i == T - 1),
                )
                nc.tensor.matmul(
                    z22_ps[:], lhsT=xb_all[:, i * D + P:(i + 1) * D],
                    rhs=g_ch[:, j * D + P:(j + 1) * D],
                    start=(i == 0), stop=(i == T - 1),
                )

        # ---- zw = z @ W using symmetry (z21 = z12^T) ----
        nc.scalar.copy(z_top_sb[:], z_top_ps[:])
        nc.vector.tensor_copy(z22_sb[:], z22_ps[:])
        zT_ps = psW.tile([P, P], BF16, tag="pw", name="zT")
        nc.tensor.transpose(zT_ps[:], z_top_sb[:, P:D], ident_bf[:])
        nc.vector.tensor_copy(z21_sb[:], zT_ps[:])

        # zw_bot first: it needs no transpose, so it overlaps the z12
        # transpose chain that zw_top depends on
        for half, (lhs1, lhs2) in (
            (1, (z_top_sb[:, P:D], z22_sb[:])),    # zw_bot = z12^T W1 + z22 W2
            (0, (z_top_sb[:, 0:P], z21_sb[:])),    # zw_top = z11 W1 + (z12^T)^T W2
        ):
            zw_ps = psW.tile([P, D], F32, tag="pw", name=f"zw{half}")
            nc.tensor.matmul(zw_ps[:], lhsT=lhs1, rhs=Wb_sb[:, 0:D],
                             start=True, stop=False)
            nc.tensor.matmul(zw_ps[:], lhsT=lhs2, rhs=Wb_sb[:, D:2 * D],
                             start=False, stop=True)
            nc.vector.tensor_copy(zw_sb[:, half * D:(half + 1) * D], zw_ps[:])

        # ---- final: out_j = f_j * (x_j @ zw) for the 8 local tiles ----
        # scales land in one persistent tile (no buffer-rotation gating);
        # 8 per-tile stores run on parallel DMA queues
        out3 = out.rearrange("(p j) d -> p j d", p=P)
        o_all = pp.tile([P, TL * D], F32)
        for i in range(TL):
            o_ps = psW.tile([P, D], F32, tag="pw", name=f"o{i}")
            for h in (1, 0):
                nc.tensor.matmul(
                    o_ps[:], lhsT=xbT[:, (2 * i + h) * P:(2 * i + h + 1) * P],
                    rhs=zw_sb[:, h * D:(h + 1) * D],
                    start=(h == 1), stop=(h == 0),
                )
            o_i = o_all[:, i * D:(i + 1) * D]
            if i % 2 == 0:
                nc.scalar.mul(o_i, o_ps[:], f_t[:, i:i + 1])
            else:
                nc.vector.tensor_scalar_mul(o_i, o_ps[:], f_t[:, i:i + 1])
            if i % 2 == 1:
                nc.sync.dma_start(
                    out3[:, i - 1:i + 1, :],
                    o_all[:, (i - 1) * D:(i + 1) * D].rearrange(
                        "p (j d) -> p j d", j=2),
                )


def _build():
    nc = bacc.Bacc("TRN2", target_bir_lowering=False, debug=False, num_devices=R)
    x = nc.dram_tensor("x", [N, D], F32, kind="ExternalInput")
    W = nc.dram_tensor("W", [D, D], F32, kind="ExternalInput")
    out = nc.dram_tensor("out", [NL, D], F32, kind="ExternalOutput")
    with nc.allow_low_precision("bf16 row reductions; validated ~1.6e-3 rel err"):
        with tile.TileContext(nc) as tc:
            _program(
                tc,
                x.ap() if hasattr(x, "ap") else x,
                W.ap() if hasattr(W, "ap") else W,
                out.ap() if hasattr(out, "ap") else out,
            )
    nc.finalize()
    return nc


def _run(inputs, trace=False):
    if "nc" not in _cache:
        _cache["nc"] = _build()
    nc = _cache["nc"]
    x = np.ascontiguousarray(inputs["x"], dtype=np.float32)
    W = np.ascontiguousarray(inputs["W"], dtype=np.float32)
    in_maps = [
        {"x": np.roll(x, -r * NL, axis=0), "W": W} for r in range(R)
    ]
    res = bass_utils.run_bass_kernel_spmd(
        nc, in_maps, core_ids=list(range(R)), trace=trace,
    )
    out = np.concatenate([res.results[r]["out"] for r in range(R)], axis=0)
    return out, res


def kernel(**inputs) -> np.ndarray:
    out, _ = _run(inputs, trace=False)
    return out
